# revision 7
# baseline (speedup 1.0000x reference)
"""Trainium2 Bass kernel for nn_AdjointODEBlock: RK4 integration of
f(h) = tanh(h @ W1 + b1) @ W2 + b2, 10 steps, dt = 0.1.

Full inputs: h (16384, 1024) f32, W1 (1024, 2048), b1 (2048,),
W2 (2048, 1024), b2 (1024,).  Data-parallel over 8 NeuronCores: the batch
dim of h is sharded 8 x 2048, the MLP params are replicated, no cross-core
communication.

Per-core layout: activations live transposed in SBUF (features on
partitions, batch on the free dim) so both weight matrices serve as the
stationary matmul operand in natural layout.  The 2048-row shard is
processed in 4 column chunks of 512; each chunk runs all 10 RK4 steps
entirely in SBUF (state never round-trips DRAM).  PE transposes convert
(B,D) <-> (D,B) at entry/exit.  Matmuls run in bf16 with fp32 PSUM
accumulation; the RK4 state and all state updates stay fp32 on the vector
engine.  The scalar engine does only Tanh (single activation table set).
"""
import sys

if "/opt/trn_rl_repo" not in sys.path:
    sys.path.insert(0, "/opt/trn_rl_repo")

import contextlib
import numpy as np

import concourse.bass as bass  # noqa: F401
import concourse.tile as tile
from concourse import mybir, bacc
from concourse.bass_utils import run_bass_kernel_spmd
from concourse.masks import make_identity

P = 128
D, HD = 1024, 2048
KD, MH = D // P, HD // P  # 8, 16
N_CORES = 8
B_FULL = 16384
B_SHARD = B_FULL // N_CORES  # 2048
BC = 512
NBC = B_SHARD // BC
NBT = BC // P
NSTEPS = 10
DT = (1.0 - 0.0) / NSTEPS

f32 = mybir.dt.float32
bf16 = mybir.dt.bfloat16
ALU = mybir.AluOpType
ACT_TANH = mybir.ActivationFunctionType.Tanh

# a_next = h + c*k ;  h_next = h + sum_ev w*k
C_EV = (DT / 2, DT / 2, DT, DT / 6)
W_EV = (DT / 6, DT / 3, DT / 3, DT / 6)


def _build():
    nc = bacc.Bacc(trn_type="TRN2", target_bir_lowering=False, debug=False,
                   num_devices=N_CORES)
    h_in = nc.declare_dram_parameter("h", [B_SHARD, D], f32, isOutput=False)
    w1_d = nc.declare_dram_parameter("W1", [D, HD], f32, isOutput=False)
    b1_d = nc.declare_dram_parameter("b1", [HD], f32, isOutput=False)
    w2_d = nc.declare_dram_parameter("W2", [HD, D], f32, isOutput=False)
    b2_d = nc.declare_dram_parameter("b2", [D], f32, isOutput=False)
    out_d = nc.declare_dram_parameter("out", [B_SHARD, D], f32, isOutput=True)

    with tile.TileContext(nc) as tc, contextlib.ExitStack() as ctx:
        const = ctx.enter_context(tc.tile_pool(name="const", bufs=1))

        def load_weight(dram, ktiles, n, tag):
            """DRAM (K, N) fp32 -> SBUF [P, ktiles, n] bf16 via staged casts."""
            wt = const.tile([P, ktiles, n], bf16, tag=tag)
            src = dram.ap().rearrange("(k p) n -> p k n", p=P)
            with tc.tile_pool(name="wstage", bufs=4) as ws:
                for k in range(ktiles):
                    stg = ws.tile([P, n], f32)
                    nc.sync.dma_start(stg[:], src[:, k, :])
                    nc.vector.tensor_copy(wt[:, k, :], stg[:])
            return wt

        w1_sb = load_weight(w1_d, KD, HD, "w1sb")
        w2_sb = load_weight(w2_d, MH, D, "w2sb")
        b1_sb = const.tile([P, MH], f32)
        nc.sync.dma_start(b1_sb[:], b1_d.ap().rearrange("(m p) -> p m", p=P))
        b2_sb = const.tile([P, KD], f32)
        nc.sync.dma_start(b2_sb[:], b2_d.ap().rearrange("(m p) -> p m", p=P))
        ident = const.tile([P, P], f32)
        make_identity(nc, ident[:])

        hpool = ctx.enter_context(tc.tile_pool(name="hstate", bufs=2))
        atpool = ctx.enter_context(tc.tile_pool(name="at", bufs=1))
        abfpool = ctx.enter_context(tc.tile_pool(name="abf", bufs=4))
        zpool = ctx.enter_context(tc.tile_pool(name="z", bufs=1))
        trpool = ctx.enter_context(tc.tile_pool(name="tr", bufs=3))
        onpool = ctx.enter_context(tc.tile_pool(name="onat", bufs=2))
        ps1p = ctx.enter_context(tc.tile_pool(name="ps1", bufs=3, space="PSUM"))
        ps2p = ctx.enter_context(tc.tile_pool(name="ps2", bufs=3, space="PSUM"))
        pstp = ctx.enter_context(tc.tile_pool(name="pst", bufs=2, space="PSUM"))

        for ibc in range(NBC):
            col0 = ibc * BC
            # entry transpose: h_in rows [col0, col0+BC) -> h_cur[d, j]
            # (cast to the bf16 matmul copy per block so step 0 never waits
            # on a monolithic copy)
            h_cur = hpool.tile([P, KD, BC], f32, tag="hstate")
            hbf = abfpool.tile([P, KD, BC], bf16, tag="abf")
            for bt in range(NBT):
                hn = trpool.tile([P, D], f32, tag="hn")
                nc.sync.dma_start(hn[:], h_in.ap()[col0 + bt * P: col0 + (bt + 1) * P, :])
                for dt_ in range(KD):
                    pst = pstp.tile([P, P], f32)
                    nc.tensor.transpose(pst[:], hn[:, dt_ * P:(dt_ + 1) * P], ident[:])
                    nc.vector.tensor_copy(h_cur[:, dt_, bt * P:(bt + 1) * P], pst[:])
                    nc.vector.tensor_copy(hbf[:, dt_, bt * P:(bt + 1) * P], pst[:])

            for s in range(NSTEPS):
                h_nxt = hpool.tile([P, KD, BC], f32, tag="hstate")
                hbf_nxt = (abfpool.tile([P, KD, BC], bf16, tag="abf",
                                        name="hbf_nxt")
                           if s < NSTEPS - 1 else None)
                a_mm = None
                for ev in range(4):
                    rhs = hbf if ev == 0 else a_mm
                    z = zpool.tile([P, MH, BC], bf16, tag="z")
                    for mh in range(MH):
                        ps1 = ps1p.tile([P, BC], f32)
                        for kd in range(KD):
                            nc.tensor.matmul(
                                ps1[:], w1_sb[:, kd, mh * P:(mh + 1) * P],
                                rhs[:, kd, :],
                                start=(kd == 0), stop=(kd == KD - 1))
                        nc.scalar.activation(z[:, mh, :], ps1[:], ACT_TANH,
                                             bias=b1_sb[:, mh:mh + 1], scale=1.0)
                    # evacuation (fp32 state updates on DVE):
                    #   t   = h + c*(ps2 + b2)      -> bf16 copy feeds next L1
                    #   tw  = w*(ps2 + b2);  h_nxt = (h|h_nxt) + tw
                    t = atpool.tile([P, KD, BC], f32, tag="at")
                    tw = atpool.tile([P, KD, BC], f32, tag="tw")
                    abf = (abfpool.tile([P, KD, BC], bf16, tag="abf", name="abf")
                           if ev < 3 else None)
                    for md in range(KD):
                        ps2 = ps2p.tile([P, BC], f32)
                        for kh in range(MH):
                            nc.tensor.matmul(
                                ps2[:], w2_sb[:, kh, md * P:(md + 1) * P],
                                z[:, kh, :],
                                start=(kh == 0), stop=(kh == MH - 1))
                        if ev < 3:
                            nc.vector.tensor_scalar(
                                t[:, md, :], ps2[:], b2_sb[:, md:md + 1],
                                C_EV[ev], ALU.add, ALU.mult)
                            nc.vector.tensor_tensor(
                                t[:, md, :], t[:, md, :], h_cur[:, md, :], ALU.add)
                            nc.vector.tensor_copy(abf[:, md, :], t[:, md, :])
                        nc.vector.tensor_scalar(
                            tw[:, md, :], ps2[:], b2_sb[:, md:md + 1],
                            W_EV[ev], ALU.add, ALU.mult)
                        nc.vector.tensor_tensor(
                            h_nxt[:, md, :], tw[:, md, :],
                            (h_cur if ev == 0 else h_nxt)[:, md, :], ALU.add)
                        if ev == 3 and hbf_nxt is not None:
                            # per-slice bf16 cast: next step's first matmuls
                            # only wait on their own slice, not the full state
                            nc.vector.tensor_copy(hbf_nxt[:, md, :], h_nxt[:, md, :])
                    a_mm = abf
                h_cur = h_nxt
                hbf = hbf_nxt

            # exit transpose: h_cur[d, j] -> out rows
            for bt in range(NBT):
                onat = onpool.tile([P, KD, P], f32, tag="onat")
                for dt_ in range(KD):
                    pst = pstp.tile([P, P], f32)
                    nc.tensor.transpose(pst[:], h_cur[:, dt_, bt * P:(bt + 1) * P],
                                        ident[:])
                    nc.vector.tensor_copy(onat[:, dt_, :], pst[:])
                nc.sync.dma_start(
                    out_d.ap()[col0 + bt * P: col0 + (bt + 1) * P, :]
                    .rearrange("p (k q) -> p k q", k=KD),
                    onat[:])
    nc.finalize()
    return nc


_NC_CACHE = []


def kernel(h, W1, b1, W2, b2):
    h = np.ascontiguousarray(h, dtype=np.float32)
    W1 = np.ascontiguousarray(W1, dtype=np.float32)
    b1 = np.ascontiguousarray(b1, dtype=np.float32)
    W2 = np.ascontiguousarray(W2, dtype=np.float32)
    b2 = np.ascontiguousarray(b2, dtype=np.float32)
    assert h.shape == (B_FULL, D)

    if not _NC_CACHE:
        _NC_CACHE.append(_build())
    nc = _NC_CACHE[0]

    in_maps = [
        {"h": h[i * B_SHARD:(i + 1) * B_SHARD], "W1": W1, "b1": b1,
         "W2": W2, "b2": b2}
        for i in range(N_CORES)
    ]
    res = run_bass_kernel_spmd(nc, in_maps, list(range(N_CORES)))
    return np.concatenate([res.results[i]["out"] for i in range(N_CORES)], axis=0)
